# revision 22
# baseline (speedup 1.0000x reference)
"""Trainium2 Bass kernel for DirectedGraphConv.

Reference math (per batch b, node n):
    out = feature + einsum("bni,doi->bno", feature, weights) + bias[graph].sum(axis=2)

Key identities used:
  * einsum sums over BOTH directions d and input dim i, so it equals
    F @ (W0 + W1)^T.  The "+ feature" term folds in as +Identity:
        out_mm = F @ (W0 + W1 + I)^T
  * bias[graph].sum(axis=2) only depends on the per-row label histogram:
        Count[bn, l] = #{m : graph[bn, m] == l}   (16 labels)
        bias_term    = Count @ bias               ([BN,16] @ [16,512])

Sharding: data-parallel over batch; 32 batches -> 4 per NeuronCore x 8 cores.
weights/bias replicated.  Each core runs an identical program (SPMD).

Performance notes:
  * fp32 matmuls on TRN2 run as 2 half-rate passes (4 cyc/row); float32r
    (fp32 data rounded to the PE's paired-bf16 grid, ~2^-16 relative) streams
    1 cyc/row when the moving free dim >= 256.  The main matmuls consume
    f32r tiles produced by the PSUM->SBUF copies (which perform the rounding).
  * Transposes (F, W', graph) are PE transpose-mode matmuls against an
    on-chip identity (single pass); the +I fold-in accumulates in PSUM.
  * Graph/histogram pipeline runs in bf16 (labels 0..15 are exact): DVE
    is_equal compares, single-pass bf16 count matmuls with one-hot selector
    stationaries accumulating CountT[16, bn] in one PSUM bank.
  * A burst of dependency-free warm-up matmuls runs during the input DMA
    window so the PE HAM clock-gate reaches 2.4 GHz before real matmuls
    start.
  * W streams per row-block (4 DMAs) and the W0+W1 sum runs off the PE
    (DVE/GpSimd) as blocks land.
"""

import numpy as np

B, N, D = 32, 128, 512
DIR = 2
L = 16  # num labels
NC = 8  # neuron cores
BPC = B // NC  # batches per core = 4
BN = BPC * N  # rows per core = 512
P = 128
WARMUP_MMS = 32

_prog_cache: dict = {}


def _build(w_stride: int):
    """Build the per-core Bass program.

    w_stride: 1 if graph arrives as int32 [BPC,N,N], 2 if it arrives as an
    int64 tensor viewed as int32 pairs [BPC,N,2N] (little-endian low word
    at even indices).
    """
    import concourse.bass as bass  # noqa: F401
    import concourse.mybir as mybir
    import concourse.tile as tile
    from concourse import bacc
    from concourse.masks import make_identity

    f32 = mybir.dt.float32
    f32r = mybir.dt.float32r
    bf16 = mybir.dt.bfloat16
    i32 = mybir.dt.int32

    nc = bacc.Bacc(
        "TRN2",
        target_bir_lowering=False,
        debug=False,
        num_devices=NC,
    )

    feat = nc.dram_tensor("feature", [BPC, N, D], f32, kind="ExternalInput").ap()
    graph = nc.dram_tensor(
        "graph", [BPC, N, N * w_stride], i32, kind="ExternalInput"
    ).ap()
    wts = nc.dram_tensor("weights", [DIR, D, D], f32, kind="ExternalInput").ap()
    bias = nc.dram_tensor("bias", [L, D], f32, kind="ExternalInput").ap()
    out = nc.dram_tensor("out", [BPC, N, D], f32, kind="ExternalOutput").ap()

    KC = D // P  # 4 k-chunks

    with tile.TileContext(nc) as tc:
        with (
            tc.tile_pool(name="const", bufs=1) as cpool,
            tc.tile_pool(name="work", bufs=1) as wpool,
            tc.tile_pool(name="psum", bufs=1, space="PSUM") as ppool,
        ):
            # ---- constants built on-chip (gpsimd) ----
            ident = cpool.tile([P, P], f32)
            make_identity(nc, ident)
            ident_bf = cpool.tile([P, P], bf16)
            make_identity(nc, ident_bf)
            # esel[m, 16*l + j] = 1.0 iff j == l  (label-selector stationaries)
            esel = cpool.tile([P, L * L], bf16)
            nc.gpsimd.memset(esel, 0.0)
            esel3 = esel.rearrange("p (l j) -> p l j", l=L)
            nc.gpsimd.affine_select(
                out=esel3,
                in_=esel3,
                compare_op=mybir.AluOpType.not_equal,
                fill=1.0,
                base=0,
                pattern=[[1, L], [-1, L]],
                channel_multiplier=0,
            )

            # ---- ACT activation-table preload (first Copy loads the table) ----
            act_warm = cpool.tile([P, 2], f32)
            nc.scalar.copy(out=act_warm[:, 0:1], in_=ident[:, 0:1])

            # ---- HAM warm-up: dependency-free matmuls during the DMA wait ----
            psum_warm = ppool.tile([P, BN], f32, tag="small", bufs=1)
            warm_rhs = cpool.tile([P, BN], bf16)
            nc.gpsimd.memset(warm_rhs, 0.0)
            for _ in range(8):
                nc.tensor.matmul(
                    out=psum_warm,
                    lhsT=ident_bf,
                    rhs=warm_rhs,
                    start=True,
                    stop=True,
                )

            # ---- DMA inputs ----
            # sync ring: graph, feature, weights (dependency-chain order);
            # scalar ring: bias
            G_sb = wpool.tile([P, BPC, N * w_stride], i32)
            nc.sync.dma_start(out=G_sb, in_=graph.rearrange("b n w -> n b w"))

            F_sb = wpool.tile([P, BPC, D], f32)
            feat_r = feat.rearrange("b n d -> n b d")
            nc.sync.dma_start(out=F_sb[:, 0:2, :], in_=feat_r[:, 0:2, :])
            nc.sync.dma_start(out=F_sb[:, 2:4, :], in_=feat_r[:, 2:4, :])

            bias_sb = wpool.tile([L, D], f32)
            nc.scalar.dma_start(out=bias_sb, in_=bias)

            W_sb = wpool.tile([P, KC, DIR, D], f32)
            for oc in range(KC):
                nc.sync.dma_start(
                    out=W_sb[:, oc, :, :],
                    in_=wts[:, oc * P : (oc + 1) * P, :].rearrange("d p i -> p d i"),
                )

            # bias rounded to the f32r grid (gpsimd; feeds the K=16 matmul)
            bias_r = wpool.tile([L, D], f32r)
            nc.gpsimd.tensor_copy(out=bias_r, in_=bias_sb)

            # W0 + W1 direction sum, per row block as W streams in (DVE/GpSimd)
            Wsum = wpool.tile([P, KC, D], f32)
            for oc in range(KC):
                eng = nc.vector if oc % 2 == 0 else nc.gpsimd
                eng.tensor_tensor(
                    out=Wsum[:, oc, :],
                    in0=W_sb[:, oc, 0, :],
                    in1=W_sb[:, oc, 1, :],
                    op=mybir.AluOpType.add,
                )

            # ---- graph: int -> bf16 (compacting cast), then PE transpose ----
            gbf = wpool.tile([P, BPC, N], bf16)
            if w_stride == 2:
                g_src = G_sb.rearrange("p b (m two) -> p b m two", two=2)[:, :, :, 0:1]
                g_dst = gbf.rearrange("p b (m one) -> p b m one", one=1)
            else:
                g_src = G_sb
                g_dst = gbf
            nc.vector.tensor_copy(out=g_dst, in_=g_src)

            psum_gt = ppool.tile([P, BN], bf16, tag="small", bufs=1)
            for c in range(BPC):
                nc.tensor.matmul(
                    out=psum_gt[:, c * P : (c + 1) * P],
                    lhsT=gbf[:, c, :],
                    rhs=ident_bf,
                    is_transpose=True,
                    start=True,
                    stop=True,
                )
            gT = wpool.tile([P, BN], bf16)  # [m, bn]
            nc.vector.tensor_copy(out=gT, in_=psum_gt)

            # ---- histogram: EQ (DVE bf16) + selector matmuls -> CountT ----
            EQ = wpool.tile([P, L, BN], bf16)
            psum_cnt = ppool.tile([L, BN], f32, tag="small", bufs=1)
            for l in range(L):
                nc.vector.tensor_scalar(
                    out=EQ[:, l, :],
                    in0=gT,
                    scalar1=float(l),
                    scalar2=None,
                    op0=mybir.AluOpType.is_equal,
                )
                nc.tensor.matmul(
                    out=psum_cnt,
                    lhsT=esel[:, l * L : (l + 1) * L],
                    rhs=EQ[:, l, :],
                    start=(l == 0),
                    stop=(l == L - 1),
                )
            cntT = wpool.tile([L, BN], f32r)
            nc.scalar.copy(out=cntT, in_=psum_cnt)

            # ---- F^T build (fp32 transpose-mode; copies round to f32r) ----
            FT = wpool.tile([P, KC, BN], f32r)  # [i', c, bn]
            for c in range(KC):
                psum_ft = ppool.tile([P, BN], f32, tag="ft", bufs=1)
                for b in range(BPC):
                    nc.tensor.matmul(
                        out=psum_ft[:, b * P : (b + 1) * P],
                        lhsT=F_sb[:, b, c * P : (c + 1) * P],
                        rhs=ident,
                        is_transpose=True,
                        start=True,
                        stop=True,
                    )
                if c % 2 == 0:
                    nc.scalar.copy(out=FT[:, c, :], in_=psum_ft)
                else:
                    nc.vector.tensor_copy(out=FT[:, c, :], in_=psum_ft)

            # ---- W'^T build: transpose-mode; +I via PSUM accum.
            # oc-outer so each W row-block is consumed as its DMA lands.
            WT = wpool.tile([P, KC, D], f32r)  # [i', c, o]
            psum_wts = [
                ppool.tile([P, D], f32, tag="wt", bufs=KC, name=f"psum_wt{c}")
                for c in range(KC)
            ]
            for oc in range(KC):
                for c in range(KC):
                    sl = slice(oc * P, (oc + 1) * P)
                    nc.tensor.matmul(
                        out=psum_wts[c][:, sl],
                        lhsT=Wsum[:, oc, c * P : (c + 1) * P],
                        rhs=ident,
                        is_transpose=True,
                        start=True,
                        stop=(oc != c),
                    )
                    if oc == c:
                        # add identity: transpose(I) = I accumulated on top
                        nc.tensor.matmul(
                            out=psum_wts[c][:, sl],
                            lhsT=ident,
                            rhs=ident,
                            is_transpose=True,
                            start=False,
                            stop=True,
                        )
            for c in range(KC):
                if c % 2 == 0:
                    nc.scalar.copy(out=WT[:, c, :], in_=psum_wts[c])
                else:
                    nc.vector.tensor_copy(out=WT[:, c, :], in_=psum_wts[c])

            # ---- main matmuls (f32r single-pass) + bias term, then store ----
            out_sb = wpool.tile([P, BPC, D], f32)
            for b in range(BPC):
                psum_out = ppool.tile([P, D], f32, tag="out", bufs=2)
                for c in range(KC):
                    nc.tensor.matmul(
                        out=psum_out,
                        lhsT=FT[:, c, b * P : (b + 1) * P],
                        rhs=WT[:, c, :],
                        start=(c == 0),
                        stop=False,
                    )
                nc.tensor.matmul(
                    out=psum_out,
                    lhsT=cntT[:, b * P : (b + 1) * P],
                    rhs=bias_r,
                    start=False,
                    stop=True,
                )
                if b == BPC - 1:
                    # last batch: split the copy across DVE+ACT and DMA per
                    # half so the exposed tail chain is shorter
                    h = D // 2
                    nc.vector.tensor_copy(
                        out=out_sb[:, b, 0:h], in_=psum_out[:, 0:h]
                    )
                    nc.scalar.copy(out=out_sb[:, b, h:D], in_=psum_out[:, h:D])
                    nc.sync.dma_start(out=out[b, :, 0:h], in_=out_sb[:, b, 0:h])
                    nc.sync.dma_start(out=out[b, :, h:D], in_=out_sb[:, b, h:D])
                else:
                    if b % 2 == 0:
                        nc.vector.tensor_copy(out=out_sb[:, b, :], in_=psum_out)
                    else:
                        nc.scalar.copy(out=out_sb[:, b, :], in_=psum_out)
                    nc.sync.dma_start(out=out[b], in_=out_sb[:, b, :])

    nc.compile()
    return nc


def _get_prog(w_stride: int):
    if w_stride not in _prog_cache:
        _prog_cache[w_stride] = _build(w_stride)
    return _prog_cache[w_stride]


def _shard_inputs(feature, graph, weights, bias):
    feature = np.ascontiguousarray(np.asarray(feature), dtype=np.float32)
    weights = np.ascontiguousarray(np.asarray(weights), dtype=np.float32)
    bias = np.ascontiguousarray(np.asarray(bias), dtype=np.float32)
    g = np.ascontiguousarray(np.asarray(graph))
    if g.dtype == np.int64:
        g32 = g.view(np.int32)  # [B, N, 2N], low word at even cols
        w_stride = 2
    elif g.dtype == np.int32:
        g32 = g
        w_stride = 1
    else:
        g32 = g.astype(np.int32)
        w_stride = 1
    in_maps = []
    for core in range(NC):
        sl = slice(core * BPC, (core + 1) * BPC)
        in_maps.append(
            {
                "feature": np.ascontiguousarray(feature[sl]),
                "graph": np.ascontiguousarray(g32[sl]),
                "weights": weights,
                "bias": bias,
            }
        )
    return in_maps, w_stride


def _run(feature, graph, weights, bias, trace=False):
    from concourse.bass_utils import run_bass_kernel_spmd

    in_maps, w_stride = _shard_inputs(feature, graph, weights, bias)
    nc = _get_prog(w_stride)
    res = run_bass_kernel_spmd(
        nc, in_maps, core_ids=list(range(NC)), trace=trace
    )
    out = np.concatenate([r["out"] for r in res.results], axis=0)
    return out, res


def kernel(feature, graph, weights, bias):
    out, _ = _run(feature, graph, weights, bias, trace=False)
    return out


# revision 23
# speedup vs baseline: 1.0272x; 1.0272x over previous
"""Trainium2 Bass kernel for DirectedGraphConv.

Reference math (per batch b, node n):
    out = feature + einsum("bni,doi->bno", feature, weights) + bias[graph].sum(axis=2)

Key identities used:
  * einsum sums over BOTH directions d and input dim i, so it equals
    F @ (W0 + W1)^T.  The "+ feature" term folds in as +Identity:
        out_mm = F @ (W0 + W1 + I)^T
  * bias[graph].sum(axis=2) only depends on the per-row label histogram:
        Count[bn, l] = #{m : graph[bn, m] == l}   (16 labels)
        bias_term    = Count @ bias               ([BN,16] @ [16,512])

Sharding: data-parallel over batch; 32 batches -> 4 per NeuronCore x 8 cores.
weights/bias replicated.  Each core runs an identical program (SPMD).

Performance notes:
  * fp32 matmuls on TRN2 run as 2 half-rate passes (4 cyc/row); float32r
    (fp32 data rounded to the PE's paired-bf16 grid, ~2^-16 relative) streams
    1 cyc/row when the moving free dim >= 256.  The main matmuls consume
    f32r tiles produced by the PSUM->SBUF copies (which perform the rounding).
  * Transposes (F, W', graph) are PE transpose-mode matmuls against an
    on-chip identity (single pass); the +I fold-in accumulates in PSUM.
  * Graph/histogram pipeline runs in bf16 (labels 0..15 are exact): DVE
    is_equal compares, single-pass bf16 count matmuls with one-hot selector
    stationaries accumulating CountT[16, bn] in one PSUM bank.
  * A burst of dependency-free warm-up matmuls runs during the input DMA
    window so the PE HAM clock-gate reaches 2.4 GHz before real matmuls
    start.
  * W streams per row-block (4 DMAs) and the W0+W1 sum runs off the PE
    (DVE/GpSimd) as blocks land.
"""

import numpy as np

B, N, D = 32, 128, 512
DIR = 2
L = 16  # num labels
NC = 8  # neuron cores
BPC = B // NC  # batches per core = 4
BN = BPC * N  # rows per core = 512
P = 128
WARMUP_MMS = 32

_prog_cache: dict = {}


def _build(w_stride: int):
    """Build the per-core Bass program.

    w_stride: 1 if graph arrives as int32 [BPC,N,N], 2 if it arrives as an
    int64 tensor viewed as int32 pairs [BPC,N,2N] (little-endian low word
    at even indices).
    """
    import concourse.bass as bass  # noqa: F401
    import concourse.mybir as mybir
    import concourse.tile as tile
    from concourse import bacc
    from concourse.masks import make_identity

    f32 = mybir.dt.float32
    f32r = mybir.dt.float32r
    bf16 = mybir.dt.bfloat16
    i32 = mybir.dt.int32

    nc = bacc.Bacc(
        "TRN2",
        target_bir_lowering=False,
        debug=False,
        num_devices=NC,
    )

    feat = nc.dram_tensor("feature", [BPC, N, D], f32, kind="ExternalInput").ap()
    graph = nc.dram_tensor(
        "graph", [BPC, N, N * w_stride], i32, kind="ExternalInput"
    ).ap()
    wts = nc.dram_tensor("weights", [DIR, D, D], f32, kind="ExternalInput").ap()
    bias = nc.dram_tensor("bias", [L, D], f32, kind="ExternalInput").ap()
    out = nc.dram_tensor("out", [BPC, N, D], f32, kind="ExternalOutput").ap()

    KC = D // P  # 4 k-chunks

    with tile.TileContext(nc) as tc:
        with (
            tc.tile_pool(name="const", bufs=1) as cpool,
            tc.tile_pool(name="work", bufs=1) as wpool,
            tc.tile_pool(name="psum", bufs=1, space="PSUM") as ppool,
        ):
            # ---- constants built on-chip (gpsimd) ----
            ident = cpool.tile([P, P], f32)
            make_identity(nc, ident)
            ident_bf = cpool.tile([P, P], bf16)
            make_identity(nc, ident_bf)
            # esel[m, 16*l + j] = 1.0 iff j == l  (label-selector stationaries)
            esel = cpool.tile([P, L * L], bf16)
            nc.gpsimd.memset(esel, 0.0)
            esel3 = esel.rearrange("p (l j) -> p l j", l=L)
            nc.gpsimd.affine_select(
                out=esel3,
                in_=esel3,
                compare_op=mybir.AluOpType.not_equal,
                fill=1.0,
                base=0,
                pattern=[[1, L], [-1, L]],
                channel_multiplier=0,
            )

            # ---- ACT activation-table preload (first Copy loads the table) ----
            act_warm = cpool.tile([P, 2], f32)
            nc.scalar.copy(out=act_warm[:, 0:1], in_=ident[:, 0:1])

            # ---- HAM warm-up: dependency-free matmuls during the DMA wait ----
            psum_warm = ppool.tile([P, P], f32, tag="small", bufs=1)
            for _ in range(WARMUP_MMS):
                nc.tensor.matmul(
                    out=psum_warm,
                    lhsT=ident_bf,
                    rhs=ident_bf,
                    start=True,
                    stop=True,
                )

            # ---- DMA inputs ----
            # sync ring: graph, feature, weights (dependency-chain order);
            # scalar ring: bias
            G_sb = wpool.tile([P, BPC, N * w_stride], i32)
            nc.sync.dma_start(out=G_sb, in_=graph.rearrange("b n w -> n b w"))

            F_sb = wpool.tile([P, BPC, D], f32)
            nc.sync.dma_start(out=F_sb, in_=feat.rearrange("b n d -> n b d"))

            bias_sb = wpool.tile([L, D], f32)
            nc.scalar.dma_start(out=bias_sb, in_=bias)

            W_sb = wpool.tile([P, KC, DIR, D], f32)
            for oc in range(KC):
                nc.sync.dma_start(
                    out=W_sb[:, oc, :, :],
                    in_=wts[:, oc * P : (oc + 1) * P, :].rearrange("d p i -> p d i"),
                )

            # bias rounded to the f32r grid (gpsimd; feeds the K=16 matmul)
            bias_r = wpool.tile([L, D], f32r)
            nc.gpsimd.tensor_copy(out=bias_r, in_=bias_sb)

            # W0 + W1 direction sum, per row block as W streams in (DVE/GpSimd)
            Wsum = wpool.tile([P, KC, D], f32)
            for oc in range(KC):
                eng = nc.vector if oc % 2 == 0 else nc.gpsimd
                eng.tensor_tensor(
                    out=Wsum[:, oc, :],
                    in0=W_sb[:, oc, 0, :],
                    in1=W_sb[:, oc, 1, :],
                    op=mybir.AluOpType.add,
                )

            # ---- graph: int -> bf16 (compacting cast), then PE transpose ----
            gbf = wpool.tile([P, BPC, N], bf16)
            if w_stride == 2:
                g_src = G_sb.rearrange("p b (m two) -> p b m two", two=2)[:, :, :, 0:1]
                g_dst = gbf.rearrange("p b (m one) -> p b m one", one=1)
            else:
                g_src = G_sb
                g_dst = gbf
            nc.vector.tensor_copy(out=g_dst, in_=g_src)

            psum_gt = ppool.tile([P, BN], bf16, tag="small", bufs=1)
            for c in range(BPC):
                nc.tensor.matmul(
                    out=psum_gt[:, c * P : (c + 1) * P],
                    lhsT=gbf[:, c, :],
                    rhs=ident_bf,
                    is_transpose=True,
                    start=True,
                    stop=True,
                )
            gT = wpool.tile([P, BN], bf16)  # [m, bn]
            nc.vector.tensor_copy(out=gT, in_=psum_gt)

            # ---- histogram: EQ (DVE bf16) + selector matmuls -> CountT ----
            EQ = wpool.tile([P, L, BN], bf16)
            psum_cnt = ppool.tile([L, BN], f32, tag="small", bufs=1)
            for l in range(L):
                nc.vector.tensor_scalar(
                    out=EQ[:, l, :],
                    in0=gT,
                    scalar1=float(l),
                    scalar2=None,
                    op0=mybir.AluOpType.is_equal,
                )
                nc.tensor.matmul(
                    out=psum_cnt,
                    lhsT=esel[:, l * L : (l + 1) * L],
                    rhs=EQ[:, l, :],
                    start=(l == 0),
                    stop=(l == L - 1),
                )
            cntT = wpool.tile([L, BN], f32r)
            nc.scalar.copy(out=cntT, in_=psum_cnt)

            # ---- F^T build (fp32 transpose-mode; copies round to f32r) ----
            FT = wpool.tile([P, KC, BN], f32r)  # [i', c, bn]
            for c in range(KC):
                psum_ft = ppool.tile([P, BN], f32, tag="ft", bufs=1)
                for b in range(BPC):
                    nc.tensor.matmul(
                        out=psum_ft[:, b * P : (b + 1) * P],
                        lhsT=F_sb[:, b, c * P : (c + 1) * P],
                        rhs=ident,
                        is_transpose=True,
                        start=True,
                        stop=True,
                    )
                if c % 2 == 0:
                    nc.scalar.copy(out=FT[:, c, :], in_=psum_ft)
                else:
                    nc.vector.tensor_copy(out=FT[:, c, :], in_=psum_ft)

            # ---- W'^T build: transpose-mode; +I via PSUM accum.
            # oc-outer so each W row-block is consumed as its DMA lands.
            WT = wpool.tile([P, KC, D], f32r)  # [i', c, o]
            psum_wts = [
                ppool.tile([P, D], f32, tag="wt", bufs=KC, name=f"psum_wt{c}")
                for c in range(KC)
            ]
            for oc in range(KC):
                for c in range(KC):
                    sl = slice(oc * P, (oc + 1) * P)
                    nc.tensor.matmul(
                        out=psum_wts[c][:, sl],
                        lhsT=Wsum[:, oc, c * P : (c + 1) * P],
                        rhs=ident,
                        is_transpose=True,
                        start=True,
                        stop=(oc != c),
                    )
                    if oc == c:
                        # add identity: transpose(I) = I accumulated on top
                        nc.tensor.matmul(
                            out=psum_wts[c][:, sl],
                            lhsT=ident,
                            rhs=ident,
                            is_transpose=True,
                            start=False,
                            stop=True,
                        )
            for c in range(KC):
                if c % 2 == 0:
                    nc.scalar.copy(out=WT[:, c, :], in_=psum_wts[c])
                else:
                    nc.vector.tensor_copy(out=WT[:, c, :], in_=psum_wts[c])

            # ---- main matmuls (f32r single-pass) + bias term, then store ----
            out_sb = wpool.tile([P, BPC, D], f32)
            for b in range(BPC):
                psum_out = ppool.tile([P, D], f32, tag="out", bufs=2)
                for c in range(KC):
                    nc.tensor.matmul(
                        out=psum_out,
                        lhsT=FT[:, c, b * P : (b + 1) * P],
                        rhs=WT[:, c, :],
                        start=(c == 0),
                        stop=False,
                    )
                nc.tensor.matmul(
                    out=psum_out,
                    lhsT=cntT[:, b * P : (b + 1) * P],
                    rhs=bias_r,
                    start=False,
                    stop=True,
                )
                if b == BPC - 1:
                    # last batch: split the copy across DVE+ACT and DMA per
                    # half so the exposed tail chain is shorter
                    h = D // 2
                    nc.vector.tensor_copy(
                        out=out_sb[:, b, 0:h], in_=psum_out[:, 0:h]
                    )
                    nc.scalar.copy(out=out_sb[:, b, h:D], in_=psum_out[:, h:D])
                    nc.sync.dma_start(out=out[b, :, 0:h], in_=out_sb[:, b, 0:h])
                    nc.sync.dma_start(out=out[b, :, h:D], in_=out_sb[:, b, h:D])
                else:
                    if b % 2 == 0:
                        nc.vector.tensor_copy(out=out_sb[:, b, :], in_=psum_out)
                    else:
                        nc.scalar.copy(out=out_sb[:, b, :], in_=psum_out)
                    nc.sync.dma_start(out=out[b], in_=out_sb[:, b, :])

    nc.compile()
    return nc


def _get_prog(w_stride: int):
    if w_stride not in _prog_cache:
        _prog_cache[w_stride] = _build(w_stride)
    return _prog_cache[w_stride]


def _shard_inputs(feature, graph, weights, bias):
    feature = np.ascontiguousarray(np.asarray(feature), dtype=np.float32)
    weights = np.ascontiguousarray(np.asarray(weights), dtype=np.float32)
    bias = np.ascontiguousarray(np.asarray(bias), dtype=np.float32)
    g = np.ascontiguousarray(np.asarray(graph))
    if g.dtype == np.int64:
        g32 = g.view(np.int32)  # [B, N, 2N], low word at even cols
        w_stride = 2
    elif g.dtype == np.int32:
        g32 = g
        w_stride = 1
    else:
        g32 = g.astype(np.int32)
        w_stride = 1
    in_maps = []
    for core in range(NC):
        sl = slice(core * BPC, (core + 1) * BPC)
        in_maps.append(
            {
                "feature": np.ascontiguousarray(feature[sl]),
                "graph": np.ascontiguousarray(g32[sl]),
                "weights": weights,
                "bias": bias,
            }
        )
    return in_maps, w_stride


def _run(feature, graph, weights, bias, trace=False):
    from concourse.bass_utils import run_bass_kernel_spmd

    in_maps, w_stride = _shard_inputs(feature, graph, weights, bias)
    nc = _get_prog(w_stride)
    res = run_bass_kernel_spmd(
        nc, in_maps, core_ids=list(range(NC)), trace=trace
    )
    out = np.concatenate([r["out"] for r in res.results], axis=0)
    return out, res


def kernel(feature, graph, weights, bias):
    out, _ = _run(feature, graph, weights, bias, trace=False)
    return out


# revision 24
# speedup vs baseline: 1.0320x; 1.0047x over previous
"""Trainium2 Bass kernel for DirectedGraphConv.

Reference math (per batch b, node n):
    out = feature + einsum("bni,doi->bno", feature, weights) + bias[graph].sum(axis=2)

Key identities used:
  * einsum sums over BOTH directions d and input dim i, so it equals
    F @ (W0 + W1)^T.  The "+ feature" term folds in as +Identity:
        out_mm = F @ (W0 + W1 + I)^T
  * bias[graph].sum(axis=2) only depends on the per-row label histogram:
        Count[bn, l] = #{m : graph[bn, m] == l}   (16 labels)
        bias_term    = Count @ bias               ([BN,16] @ [16,512])

Sharding: data-parallel over batch; 32 batches -> 4 per NeuronCore x 8 cores.
weights/bias replicated.  Each core runs an identical program (SPMD).

Performance notes:
  * fp32 matmuls on TRN2 run as 2 half-rate passes (4 cyc/row); float32r
    (fp32 data rounded to the PE's paired-bf16 grid, ~2^-16 relative) streams
    1 cyc/row when the moving free dim >= 256.  The main matmuls consume
    f32r tiles produced by the PSUM->SBUF copies (which perform the rounding).
  * Transposes (F, W', graph) are PE transpose-mode matmuls against an
    on-chip identity (single pass); the +I fold-in accumulates in PSUM.
  * Graph/histogram pipeline runs in bf16 (labels 0..15 are exact): DVE
    is_equal compares, single-pass bf16 count matmuls with one-hot selector
    stationaries accumulating CountT[16, bn] in one PSUM bank.
  * A burst of dependency-free warm-up matmuls runs during the input DMA
    window so the PE HAM clock-gate reaches 2.4 GHz before real matmuls
    start.
  * W streams per row-block (4 DMAs) and the W0+W1 sum runs off the PE
    (DVE/GpSimd) as blocks land.
"""

import numpy as np

B, N, D = 32, 128, 512
DIR = 2
L = 16  # num labels
NC = 8  # neuron cores
BPC = B // NC  # batches per core = 4
BN = BPC * N  # rows per core = 512
P = 128
WARMUP_MMS = 32

_prog_cache: dict = {}


def _build(w_stride: int):
    """Build the per-core Bass program.

    w_stride: 1 if graph arrives as int32 [BPC,N,N], 2 if it arrives as an
    int64 tensor viewed as int32 pairs [BPC,N,2N] (little-endian low word
    at even indices).
    """
    import concourse.bass as bass  # noqa: F401
    import concourse.mybir as mybir
    import concourse.tile as tile
    from concourse import bacc
    from concourse.masks import make_identity

    f32 = mybir.dt.float32
    f32r = mybir.dt.float32r
    bf16 = mybir.dt.bfloat16
    i32 = mybir.dt.int32

    nc = bacc.Bacc(
        "TRN2",
        target_bir_lowering=False,
        debug=False,
        num_devices=NC,
    )

    feat = nc.dram_tensor("feature", [BPC, N, D], f32, kind="ExternalInput").ap()
    graph = nc.dram_tensor(
        "graph", [BPC, N, N * w_stride], i32, kind="ExternalInput"
    ).ap()
    wts = nc.dram_tensor("weights", [DIR, D, D], f32, kind="ExternalInput").ap()
    bias = nc.dram_tensor("bias", [L, D], f32, kind="ExternalInput").ap()
    out = nc.dram_tensor("out", [BPC, N, D], f32, kind="ExternalOutput").ap()

    KC = D // P  # 4 k-chunks

    with tile.TileContext(nc) as tc:
        with (
            tc.tile_pool(name="const", bufs=1) as cpool,
            tc.tile_pool(name="work", bufs=1) as wpool,
            tc.tile_pool(name="psum", bufs=1, space="PSUM") as ppool,
        ):
            # ---- constants built on-chip (gpsimd) ----
            ident = cpool.tile([P, P], f32)
            make_identity(nc, ident)
            ident_bf = cpool.tile([P, P], bf16)
            make_identity(nc, ident_bf)
            # esel[m, 16*l + j] = 1.0 iff j == l  (label-selector stationaries)
            esel = cpool.tile([P, L * L], bf16)
            nc.gpsimd.memset(esel, 0.0)
            esel3 = esel.rearrange("p (l j) -> p l j", l=L)
            nc.gpsimd.affine_select(
                out=esel3,
                in_=esel3,
                compare_op=mybir.AluOpType.not_equal,
                fill=1.0,
                base=0,
                pattern=[[1, L], [-1, L]],
                channel_multiplier=0,
            )

            # ---- ACT activation-table preload (first Copy loads the table) ----
            act_warm = cpool.tile([P, 2], f32)
            nc.scalar.copy(out=act_warm[:, 0:1], in_=ident[:, 0:1])

            # ---- HAM warm-up: dependency-free matmuls during the DMA wait ----
            psum_warm = ppool.tile([P, P], f32, tag="small", bufs=1)
            for _ in range(WARMUP_MMS):
                nc.tensor.matmul(
                    out=psum_warm,
                    lhsT=ident_bf,
                    rhs=ident_bf,
                    start=True,
                    stop=True,
                )

            # ---- DMA inputs ----
            # sync ring: graph, feature, weights (dependency-chain order);
            # scalar ring: bias
            G_sb = wpool.tile([P, BPC, N * w_stride], i32)
            nc.sync.dma_start(out=G_sb, in_=graph.rearrange("b n w -> n b w"))

            F_sb = wpool.tile([P, BPC, D], f32)
            nc.sync.dma_start(out=F_sb, in_=feat.rearrange("b n d -> n b d"))

            bias_sb = wpool.tile([L, D], f32)
            nc.scalar.dma_start(out=bias_sb, in_=bias)

            W_sb = wpool.tile([P, KC, DIR, D], f32)
            for oc in range(KC):
                nc.sync.dma_start(
                    out=W_sb[:, oc, :, :],
                    in_=wts[:, oc * P : (oc + 1) * P, :].rearrange("d p i -> p d i"),
                )

            # bias rounded to the f32r grid (gpsimd; feeds the K=16 matmul)
            bias_r = wpool.tile([L, D], f32r)
            nc.gpsimd.tensor_copy(out=bias_r, in_=bias_sb)

            # W0 + W1 direction sum, per row block as W streams in (DVE/GpSimd)
            Wsum = wpool.tile([P, KC, D], f32)
            for oc in range(KC):
                eng = nc.vector if oc % 2 == 0 else nc.gpsimd
                eng.tensor_tensor(
                    out=Wsum[:, oc, :],
                    in0=W_sb[:, oc, 0, :],
                    in1=W_sb[:, oc, 1, :],
                    op=mybir.AluOpType.add,
                )

            # ---- graph: int -> bf16 (compacting cast), then PE transpose ----
            gbf = wpool.tile([P, BPC, N], bf16)
            if w_stride == 2:
                g_src = G_sb.rearrange("p b (m two) -> p b m two", two=2)[:, :, :, 0:1]
                g_dst = gbf.rearrange("p b (m one) -> p b m one", one=1)
            else:
                g_src = G_sb
                g_dst = gbf
            nc.vector.tensor_copy(out=g_dst, in_=g_src)

            psum_gt = ppool.tile([P, BN], bf16, tag="small", bufs=1)
            for c in range(BPC):
                nc.tensor.matmul(
                    out=psum_gt[:, c * P : (c + 1) * P],
                    lhsT=gbf[:, c, :],
                    rhs=ident_bf,
                    is_transpose=True,
                    start=True,
                    stop=True,
                )
            gT = wpool.tile([P, BN], bf16)  # [m, bn]
            nc.vector.tensor_copy(out=gT, in_=psum_gt)

            # ---- histogram: EQ (DVE bf16) + selector matmuls -> CountT ----
            EQ = wpool.tile([P, L, BN], bf16)
            psum_cnt = ppool.tile([L, BN], f32, tag="small", bufs=1)
            for l in range(L):
                nc.vector.tensor_scalar(
                    out=EQ[:, l, :],
                    in0=gT,
                    scalar1=float(l),
                    scalar2=None,
                    op0=mybir.AluOpType.is_equal,
                )
                nc.tensor.matmul(
                    out=psum_cnt,
                    lhsT=esel[:, l * L : (l + 1) * L],
                    rhs=EQ[:, l, :],
                    start=(l == 0),
                    stop=(l == L - 1),
                )
            cntT = wpool.tile([L, BN], f32r)
            nc.scalar.copy(out=cntT, in_=psum_cnt)

            # ---- F^T build (fp32 transpose-mode; copies round to f32r) ----
            FT = wpool.tile([P, KC, BN], f32r)  # [i', c, bn]
            for c in range(KC):
                psum_ft = ppool.tile([P, BN], f32, tag="ft", bufs=1)
                for b in range(BPC):
                    nc.tensor.matmul(
                        out=psum_ft[:, b * P : (b + 1) * P],
                        lhsT=F_sb[:, b, c * P : (c + 1) * P],
                        rhs=ident,
                        is_transpose=True,
                        start=True,
                        stop=True,
                    )
                if c % 2 == 0:
                    nc.scalar.copy(out=FT[:, c, :], in_=psum_ft)
                else:
                    nc.vector.tensor_copy(out=FT[:, c, :], in_=psum_ft)

            # ---- W'^T build: transpose-mode; +I via PSUM accum.
            # oc-outer so each W row-block is consumed as its DMA lands.
            WT = wpool.tile([P, KC, D], f32r)  # [i', c, o]
            psum_wts = [
                ppool.tile([P, D], f32, tag="wt", bufs=KC, name=f"psum_wt{c}")
                for c in range(KC)
            ]
            for oc in range(KC):
                for c in range(KC):
                    sl = slice(oc * P, (oc + 1) * P)
                    nc.tensor.matmul(
                        out=psum_wts[c][:, sl],
                        lhsT=Wsum[:, oc, c * P : (c + 1) * P],
                        rhs=ident,
                        is_transpose=True,
                        start=True,
                        stop=(oc != c),
                    )
                    if oc == c:
                        # add identity: transpose(I) = I accumulated on top
                        nc.tensor.matmul(
                            out=psum_wts[c][:, sl],
                            lhsT=ident,
                            rhs=ident,
                            is_transpose=True,
                            start=False,
                            stop=True,
                        )
            # split copies: the first 3 o-blocks of each column are ready
            # before the last W row-block lands (subtile deps), leaving only
            # a small [128,128] copy on the critical W3 chain
            h3 = 3 * P
            for c in range(KC):
                if c % 2 == 0:
                    nc.scalar.copy(out=WT[:, c, 0:h3], in_=psum_wts[c][:, 0:h3])
                else:
                    nc.vector.tensor_copy(
                        out=WT[:, c, 0:h3], in_=psum_wts[c][:, 0:h3]
                    )
            for c in range(KC):
                if c % 2 == 0:
                    nc.scalar.copy(out=WT[:, c, h3:D], in_=psum_wts[c][:, h3:D])
                else:
                    nc.vector.tensor_copy(
                        out=WT[:, c, h3:D], in_=psum_wts[c][:, h3:D]
                    )

            # ---- main matmuls (f32r single-pass) + bias term, then store ----
            out_sb = wpool.tile([P, BPC, D], f32)
            for b in range(BPC):
                psum_out = ppool.tile([P, D], f32, tag="out", bufs=2)
                for c in range(KC):
                    nc.tensor.matmul(
                        out=psum_out,
                        lhsT=FT[:, c, b * P : (b + 1) * P],
                        rhs=WT[:, c, :],
                        start=(c == 0),
                        stop=False,
                    )
                nc.tensor.matmul(
                    out=psum_out,
                    lhsT=cntT[:, b * P : (b + 1) * P],
                    rhs=bias_r,
                    start=False,
                    stop=True,
                )
                if b == BPC - 1:
                    # last batch: split the copy across DVE+ACT and DMA per
                    # half so the exposed tail chain is shorter
                    h = D // 2
                    nc.vector.tensor_copy(
                        out=out_sb[:, b, 0:h], in_=psum_out[:, 0:h]
                    )
                    nc.scalar.copy(out=out_sb[:, b, h:D], in_=psum_out[:, h:D])
                    nc.sync.dma_start(out=out[b, :, 0:h], in_=out_sb[:, b, 0:h])
                    nc.sync.dma_start(out=out[b, :, h:D], in_=out_sb[:, b, h:D])
                else:
                    if b % 2 == 0:
                        nc.vector.tensor_copy(out=out_sb[:, b, :], in_=psum_out)
                    else:
                        nc.scalar.copy(out=out_sb[:, b, :], in_=psum_out)
                    nc.sync.dma_start(out=out[b], in_=out_sb[:, b, :])

    nc.compile()
    return nc


def _get_prog(w_stride: int):
    if w_stride not in _prog_cache:
        _prog_cache[w_stride] = _build(w_stride)
    return _prog_cache[w_stride]


def _shard_inputs(feature, graph, weights, bias):
    feature = np.ascontiguousarray(np.asarray(feature), dtype=np.float32)
    weights = np.ascontiguousarray(np.asarray(weights), dtype=np.float32)
    bias = np.ascontiguousarray(np.asarray(bias), dtype=np.float32)
    g = np.ascontiguousarray(np.asarray(graph))
    if g.dtype == np.int64:
        g32 = g.view(np.int32)  # [B, N, 2N], low word at even cols
        w_stride = 2
    elif g.dtype == np.int32:
        g32 = g
        w_stride = 1
    else:
        g32 = g.astype(np.int32)
        w_stride = 1
    in_maps = []
    for core in range(NC):
        sl = slice(core * BPC, (core + 1) * BPC)
        in_maps.append(
            {
                "feature": np.ascontiguousarray(feature[sl]),
                "graph": np.ascontiguousarray(g32[sl]),
                "weights": weights,
                "bias": bias,
            }
        )
    return in_maps, w_stride


def _run(feature, graph, weights, bias, trace=False):
    from concourse.bass_utils import run_bass_kernel_spmd

    in_maps, w_stride = _shard_inputs(feature, graph, weights, bias)
    nc = _get_prog(w_stride)
    res = run_bass_kernel_spmd(
        nc, in_maps, core_ids=list(range(NC)), trace=trace
    )
    out = np.concatenate([r["out"] for r in res.results], axis=0)
    return out, res


def kernel(feature, graph, weights, bias):
    out, _ = _run(feature, graph, weights, bias, trace=False)
    return out
